# revision 1
# baseline (speedup 1.0000x reference)
"""Trainium2 Bass kernel for nn_BaconAdditionReasoner (segment_reduce).

Math (per row b of 1M):
  a = p1 @ minmax(W1); b = p2 @ minmax(W2)           # [10] each
  s_ij = min(a_i, b_j); one_minus = 1 - clip(s)       # [10,10]
  y_k  = 1 - prod_{i+j=k} one_minus_ij                # 19 anti-diag bins
  y    = y / (sum_k y_k + 1e-9)

Kernel formulation (avoids materializing min/clip and the mask matmul):
  alpha = p1 @ (1 - minmax(W1))  (rows of p1 sum to 1)  -> one_minus rows
  s_log_ij = max(ln(alpha_i), ln(beta_j))   [log is monotone; the
      reference clip at 1e-6/1-1e-6 never fires for this distribution]
  logP_k = sum over anti-diagonal (stride-9 slices of the flattened
      10x10; mirror bins k and 18-k fused into one paired reduce)
  y = (1 - exp(logP)) normalized by (19 + 1e-9 - sum exp(logP)).

Layout: batch rows on the 128 partitions, R rows per partition per
bigtile (2 warm-up tiles at R=48 for fast pipeline fill, then R=128),
rows contiguous in HBM per partition. The per-row 10x10 matmuls run on
the PE via 12-row-packed transposes (lhsT = transposed p-block, rhs =
kron(I_12, V)); Ln/Exp/copies on ACT; outer-max, paired reduces and
normalize on DVE.

Sharding: pure data parallel over 8 cores, 131072 rows each.
"""
import sys

if '/opt/trn_rl_repo' not in sys.path:
    sys.path.insert(0, '/opt/trn_rl_repo')

import numpy as np

B = 1048576
N_CORES = 8
RPC = B // N_CORES          # 131072 rows per core
P = 128                     # partitions
NT = 16                     # work units of 8192 rows (for bench scaling)

CNT = [min(k, 18 - k) + 1 for k in range(19)]
I0 = [max(0, k - 9) for k in range(19)]


def _groups_for(r):
    """r-slices per PE transpose group (12 rows of 10 -> K=120)."""
    g = [12] * (r // 12)
    if r % 12:
        g.append(r % 12)
    return g


def _schedule(nt):
    """Tile schedule: two small R=48 tiles first so the DVE phase starts
    early (short pipeline-fill), then R=128 tiles for low per-instruction
    overhead. Returns [(row0, R), ...] covering nt*8192 rows."""
    rows = nt * P * 64
    out, row0 = [], 0
    if rows >= P * 2 * 48 + P * 128:
        for _ in range(2):
            out.append((row0, 48)); row0 += P * 48
    while rows - row0 >= P * 128:
        out.append((row0, 128)); row0 += P * 128
    while rows - row0 > 0:
        r = (rows - row0) // P
        assert r > 0 and (rows - row0) % P == 0
        out.append((row0, r)); row0 += P * r
    return out

_CACHED = {}


def _build_nc(nt=NT, reps=1):
    import bass_rust as _br
    import concourse.mybir as mybir
    from concourse.bacc import Bacc
    from concourse.mybir import AluOpType
    from concourse.tile import TileContext

    F32 = mybir.dt.float32

    # Bacc (not Bass): its finalize() runs move_matmul_waits_to_ldweights +
    # generate_event_semaphores, required because walrus allows only one
    # sync wait on a self-loading fp32 Matmult.
    nc = Bacc()
    p1d = nc.dram_tensor("p1", [RPC, 10], F32, kind="ExternalInput")
    p2d = nc.dram_tensor("p2", [RPC, 10], F32, kind="ExternalInput")
    v1d = nc.dram_tensor("v1b", [120, 120], F32, kind="ExternalInput")
    v2d = nc.dram_tensor("v2b", [120, 120], F32, kind="ExternalInput")
    idd = nc.dram_tensor("ident", [128, 128], F32, kind="ExternalInput")
    yd = nc.dram_tensor("y", [RPC, 19], F32, kind="ExternalOutput")

    sched = _schedule(nt)

    with TileContext(nc) as tc:
        with (
            tc.tile_pool(name="const", bufs=1) as cpool,
            tc.tile_pool(name="io", bufs=3) as io,
            tc.tile_pool(name="ab", bufs=2) as abp,
            tc.tile_pool(name="pt", bufs=3) as ptp,
            tc.tile_pool(name="s", bufs=2) as sp,
            tc.tile_pool(name="small", bufs=3) as sm,
            tc.tile_pool(name="tp", bufs=4, space="PSUM") as tpp,
            tc.tile_pool(name="mm", bufs=4, space="PSUM") as mmp,
        ):
            v1t = cpool.tile([120, 120], F32)
            v2t = cpool.tile([120, 120], F32)
            idt = cpool.tile([128, 128], F32)
            nc.sync.dma_start(v1t[:], v1d[:])
            nc.sync.dma_start(v2t[:], v2d[:])
            nc.sync.dma_start(idt[:], idd[:])

            for row0, R in [s for _ in range(reps) for s in sched]:
                nrows = P * R
                p1v = p1d[row0:row0 + nrows, :].rearrange(
                    "(p r) c -> p (r c)", p=P)
                p2v = p2d[row0:row0 + nrows, :].rearrange(
                    "(p r) c -> p (r c)", p=P)
                yv = yd[row0:row0 + nrows, :].rearrange(
                    "(p r) k -> p (r k)", p=P)
                p1t = io.tile([P, R * 10], F32, tag="p1t")
                p2t = io.tile([P, R * 10], F32, tag="p2t")
                nc.sync.dma_start(p1t[:], p1v)
                nc.sync.dma_start(p2t[:], p2v)

                abt = abp.tile([P, R, 20], F32, tag="ab")
                r0 = 0
                for gs in _groups_for(R):
                    K = gs * 10
                    for src, vt, o in ((p1t, v1t, 0), (p2t, v2t, 10)):
                        tp = tpp.tile([K, 128], F32, tag="tp")
                        nc.tensor.transpose(
                            tp[:], src[:, r0 * 10:(r0 + gs) * 10], idt[:])
                        pt = ptp.tile([K, 128], F32, tag="pt")
                        nc.scalar.copy(pt[:], tp[:])
                        mm = mmp.tile([P, K], F32, tag="mm")
                        nc.tensor.matmul(mm[:], pt[:], vt[0:K, 0:K],
                                         start=True, stop=True)
                        # Ln fused into the PSUM->SBUF copy (Copy and Ln
                        # share activation-table sets, so no extra loads)
                        nc.scalar.activation(
                            abt[:, r0:r0 + gs, o:o + 10],
                            mm[:].rearrange("p (r c) -> p r c", c=10),
                            mybir.ActivationFunctionType.Ln)
                    r0 += gs

                lab = abt  # already log(alpha)|log(beta)

                # s_log[:, r, i, j] = max(la_i, lb_j). The reference's
                # clip to [1e-6, 1-1e-6] is omitted: alpha/beta = p @ V with
                # V minmax-normalized and p a probability row, so values sit
                # far inside (0.1, 0.9) and the clip never fires.
                st = sp.tile([P, R, 10, 10], F32, tag="s")
                lpt = sm.tile([P, R, 19], F32, tag="lp")
                # For the very first tile, emit the outer-max + reduces per
                # PE group so the DVE phase starts as soon as the first
                # 12-row group's logs land (shaves pipeline-fill); later
                # tiles use whole-tile ops for minimal instruction count.
                if row0 == 0:
                    subs, rr = [], 0
                    for gs_ in _groups_for(R):
                        subs.append((rr, gs_)); rr += gs_
                else:
                    subs = [(0, R)]
                for sr0, srn in subs:
                    sl = slice(sr0, sr0 + srn)
                    a_v = lab[:, sl, 0:10].unsqueeze(3).broadcast_to(
                        (P, srn, 10, 10))
                    b_v = lab[:, sl, 10:20].unsqueeze(2).broadcast_to(
                        (P, srn, 10, 10))
                    nc.vector.tensor_tensor(st[:, sl], a_v, b_v,
                                            AluOpType.max)
                    # anti-diagonal log-sums; mirror bins k and 18-k share a
                    # count -> one paired strided reduce:
                    # in  [P, (r), (pair=2, step 99-11k), (cnt, step 9)]
                    # out [P, (r), (pair=2, step 18-2k), 1]
                    s_flat = st[:, sl].rearrange("p r a b -> p r (a b)")
                    for k in range(10):
                        cnt = CNT[k]
                        if k == 9:
                            nc.vector.tensor_reduce(
                                lpt[:, sl, 9:10],
                                s_flat[:, :, 9:9 + 81 + 1:9],
                                axis=mybir.AxisListType.X, op=AluOpType.add)
                            continue
                        seg = (s_flat[:, :, k:k + 9 * (cnt - 1) + 1:9]
                               if cnt > 1 else s_flat[:, :, k:k + 1])
                        raw = seg.ap
                        raw.insert(2, [99 - 11 * k, 2])
                        seg2 = _br.AP(tensor=seg.tensor, offset=seg.offset,
                                      ap=raw)
                        outb = lpt[:, sl, k:k + 1]
                        raw_o = outb.ap
                        raw_o.insert(2, [18 - 2 * k, 2])
                        out2 = _br.AP(tensor=outb.tensor, offset=outb.offset,
                                      ap=raw_o)
                        nc.vector.tensor_reduce(
                            out2, seg2, axis=mybir.AxisListType.X,
                            op=AluOpType.add)

                # P = exp(logP), in place on lpt
                nc.scalar.activation(
                    lpt[:].rearrange("p r k -> p (r k)"),
                    lpt[:].rearrange("p r k -> p (r k)"),
                    mybir.ActivationFunctionType.Exp)
                # denom = 19 + 1e-9 - sum(P); r = 1/denom
                spt = sm.tile([P, R], F32, tag="S")
                nc.vector.tensor_reduce(spt[:], lpt[:],
                                        axis=mybir.AxisListType.X,
                                        op=AluOpType.add)
                nc.vector.tensor_scalar(spt[:], spt[:], -1.0, 19.0 + 1e-9,
                                        AluOpType.mult, AluOpType.add)
                rt = sm.tile([P, R], F32, tag="r")
                nc.vector.reciprocal(rt[:], spt[:])
                # u = 1 - P on ACT (in place), then y = u*r (in place)
                nc.scalar.activation(
                    lpt[:].rearrange("p r k -> p (r k)"),
                    lpt[:].rearrange("p r k -> p (r k)"),
                    mybir.ActivationFunctionType.Copy, bias=1.0, scale=-1.0)
                r_b = rt[:].unsqueeze(2).broadcast_to((P, R, 19))
                nc.vector.tensor_tensor(lpt[:], lpt[:], r_b, AluOpType.mult)
                nc.sync.dma_start(yv, lpt[:].rearrange("p r k -> p (r k)"))

    nc.finalize()
    return nc


def _host_consts(W1, W2):
    def mmn(W):
        W = W.astype(np.float32)
        lo = W.min(1, keepdims=True)
        hi = W.max(1, keepdims=True)
        return (W - lo) / (hi - lo + np.float32(1e-8))

    eye12 = np.eye(12, dtype=np.float32)
    v1b = np.kron(eye12, (np.float32(1.0) - mmn(W1))).astype(np.float32)
    v2b = np.kron(eye12, (np.float32(1.0) - mmn(W2))).astype(np.float32)
    ident = np.eye(128, dtype=np.float32)
    return v1b, v2b, ident


def kernel(p1, p2, W1, W2, mask=None, **_unused):
    from concourse.bass_utils import run_bass_kernel_spmd

    if 'nc' not in _CACHED:
        _CACHED['nc'] = _build_nc()
    nc = _CACHED['nc']

    v1b, v2b, ident = _host_consts(W1, W2)
    p1 = np.ascontiguousarray(p1, dtype=np.float32)
    p2 = np.ascontiguousarray(p2, dtype=np.float32)

    in_maps = []
    for c in range(N_CORES):
        sl = slice(c * RPC, (c + 1) * RPC)
        in_maps.append({
            "p1": p1[sl], "p2": p2[sl],
            "v1b": v1b, "v2b": v2b, "ident": ident,
        })
    res = run_bass_kernel_spmd(nc, in_maps, list(range(N_CORES)))
    out = np.concatenate([res.results[c]["y"] for c in range(N_CORES)], axis=0)
    return out.astype(np.float32)


if __name__ == "__main__":
    rng = np.random.default_rng(0)
    p1 = rng.random((B, 10), dtype=np.float32)
    p1 /= p1.sum(1, keepdims=True)
    p2 = rng.random((B, 10), dtype=np.float32)
    p2 /= p2.sum(1, keepdims=True)
    W1 = rng.random((10, 10), dtype=np.float32)
    W2 = rng.random((10, 10), dtype=np.float32)
    y = kernel(p1, p2, W1, W2)
    print("kernel ran, y shape", y.shape, "sum", float(y.sum()))



# revision 37
# speedup vs baseline: 2.5462x; 2.5462x over previous
"""Trainium2 Bass kernel for nn_BaconAdditionReasoner (segment_reduce).

Math (per row b of 1M):
  a = p1 @ minmax(W1); b = p2 @ minmax(W2)           # [10] each
  s_ij = min(a_i, b_j); one_minus = 1 - clip(s)       # [10,10]
  y_k  = 1 - prod_{i+j=k} one_minus_ij                # 19 anti-diag bins
  y    = y / (sum_k y_k + 1e-9)

Kernel formulation:
  alpha = p1 @ (1 - minmax(W1))  (rows of p1 sum to 1) -> one_minus rows
  la_i = ln alpha_i, lb_j = ln beta_j  (fp16)
  st[bin k slots] = max(la_i, lb_{k-i})  -- bin-major fp16, one DVE/Pool
      tensor_tensor per mirror-bin-pair (k and 18-k fused via a stride-2
      pair dim; fp16 packed last dims engage the DVE 2x mode)
  logP_k = in-place reversed-half fold adds over each bin's slots (fp16,
      2x), final fold level in fp32
  y = (1 - exp(logP)) / (19 + 1e-9 - sum exp(logP))

Engines: PE does per-12-row-block transposes + fp16 matmuls; ACT does the
PSUM->SBUF copies (batched), Ln, Exp and (1-P); DVE does most maxes +
folds + normalize scalars; Pool (GPSIMD) takes two max classes, the
19-bin sum and the final scale multiply.

Sharding: pure data parallel over 8 cores, 131072 rows each.
"""
import sys

if '/opt/trn_rl_repo' not in sys.path:
    sys.path.insert(0, '/opt/trn_rl_repo')

import numpy as np

B = 1048576
N_CORES = 8
RPC = B // N_CORES          # 131072 rows per core
P = 128                     # partitions
NT = 16                     # work units of 8192 rows (for bench scaling)

# class c = mirror bins (c, 18-c), cnt = c+1 slots each; class 9 = bin 9.
# st layout: bin k's region is slots [10k, 10k+cnt); after the in-place
# folds every bin holds its 2-way partial sums in slots {10k, 10k+1}
# (cnt-1 bins have a one-time-zeroed pad at 10k+1), so the final fold is
# ONE batched stride-10 tensor_tensor over all 19 bins.

# NOTE: the Pool/GPSIMD engine only encodes Add/Subtract/Multiply-type
# tensor_tensor ops in walrus codegen (max is rejected), so maxes stay on
# DVE and Pool takes add/multiply work instead.
POOL_FOLD_CLASSES = (6, 7, 8, 9)   # classes whose fold adds run on Pool
Y_ON_POOL = True            # final y = u*r multiply on Pool vs DVE
FINAL_ON_POOL = True        # batched stride-10 final fold on Pool vs DVE
SUM_L1_ON_POOL = True       # first 19-sum fold level on Pool
R_MAIN = 128                # main tile rows/partition
GROUPS_PER_BATCH = 4        # transpose/matmul groups per PSUM batch


def _groups_for(r):
    """row-group sizes per PE transpose (12 rows of 10 -> K=120)."""
    g = [12] * (r // 12)
    if r % 12:
        g.append(r % 12)
    return g


def _batches_for(r):
    """Batches of uniform-size groups for the PSUM-copy / Ln batching.
    Returns [(row0, gs, ngroups), ...]."""
    out = []
    full = r // 12
    row0 = 0
    while full > 0:
        take = min(GROUPS_PER_BATCH, full)
        out.append((row0, 12, take))
        row0 += 12 * take
        full -= take
    if r % 12:
        out.append((row0, r % 12, 1))
    return out


def _schedule(nt):
    """Small ramp-up tiles (fast pipeline fill), R_MAIN tiles in the
    middle, then small ramp-down tiles so the serial per-tile tail
    (exp -> y -> DMA) drains quickly at the end."""
    rows = nt * P * 64
    lead, trail = [24, 48, 80], [32, 16]
    out, row0 = [], 0
    budget = rows // P
    if budget >= sum(lead) + sum(trail) + R_MAIN:
        for r in lead:
            out.append((row0, r)); row0 += P * r
        budget -= sum(lead) + sum(trail)
    else:
        trail = []
    while budget >= R_MAIN:
        out.append((row0, R_MAIN)); row0 += P * R_MAIN
        budget -= R_MAIN
    if budget > 0:
        out.append((row0, budget)); row0 += P * budget
    for r in trail:
        out.append((row0, r)); row0 += P * r
    assert row0 == rows
    return out

_CACHED = {}


def _build_nc(nt=NT, reps=1):
    import bass_rust as _br
    import concourse.mybir as mybir
    from concourse.bacc import Bacc
    from concourse.mybir import AluOpType
    from concourse.tile import TileContext

    F32 = mybir.dt.float32
    F16 = mybir.dt.float16

    def with_pair(ap_view, pos, stride, n=2):
        raw = ap_view.ap
        raw.insert(pos, [stride, n])
        return _br.AP(tensor=ap_view.tensor, offset=ap_view.offset, ap=raw)

    nc = Bacc()
    p1d = nc.dram_tensor("p1", [RPC, 10], F16, kind="ExternalInput")
    p2d = nc.dram_tensor("p2", [RPC, 10], F16, kind="ExternalInput")
    v1d = nc.dram_tensor("v1b", [120, 120], F16, kind="ExternalInput")
    v2d = nc.dram_tensor("v2b", [120, 120], F16, kind="ExternalInput")
    idd = nc.dram_tensor("ident", [128, 128], F16, kind="ExternalInput")
    yd = nc.dram_tensor("y", [RPC, 19], F32, kind="ExternalOutput")

    sched = _schedule(nt)

    with TileContext(nc) as tc:
        with (
            tc.tile_pool(name="const", bufs=1) as cpool,
            tc.tile_pool(name="io", bufs=3) as io,
            tc.tile_pool(name="pt", bufs=3) as ptp,
            tc.tile_pool(name="ab", bufs=2) as abp,
            tc.tile_pool(name="st", bufs=2) as stp,
            tc.tile_pool(name="lp", bufs=3) as lpp,
            tc.tile_pool(name="w", bufs=3) as wp,
            tc.tile_pool(name="sm", bufs=3) as sm,
            tc.tile_pool(name="tp", bufs=2, space="PSUM") as tpp,
            tc.tile_pool(name="mm", bufs=2, space="PSUM") as mmp,
        ):
            v1t = cpool.tile([120, 120], F16)
            v2t = cpool.tile([120, 120], F16)
            idt = cpool.tile([128, 128], F16)
            nc.sync.dma_start(v1t[:], v1d[:])
            nc.sync.dma_start(v2t[:], v2d[:])
            nc.sync.dma_start(idt[:], idd[:])
            # All ACT funcs used (Ln, Exp, Copy) live in act-table set 6
            # (natural_log_exp_and_others); pin it once instead of letting
            # the auto-pass thrash between the Ln-only and Exp-only sets.
            nc.scalar.add_instruction(mybir.InstLoadActFuncSet(
                name="manual_actload0", act_func_set_id=6))
            # one-time zero of the pad slots (10k+1 for cnt-1 bins k=0,18)
            # in both rotating st buffers; in-loop ops never write them
            for _ in range(2):
                stz = stp.tile([P, R_MAIN, 190], F16, tag="st")
                nc.vector.memset(stz[:, :, 1:2], 0.0)
                nc.vector.memset(stz[:, :, 181:182], 0.0)

            for row0, R in [s for _ in range(reps) for s in sched]:
                nrows = P * R
                p1v = p1d[row0:row0 + nrows, :].rearrange(
                    "(p r) c -> p (r c)", p=P)
                p2v = p2d[row0:row0 + nrows, :].rearrange(
                    "(p r) c -> p (r c)", p=P)
                yv = yd[row0:row0 + nrows, :].rearrange(
                    "(p r) k -> p (r k)", p=P)
                p1t = io.tile([P, R * 10], F16, tag="p1t")
                p2t = io.tile([P, R * 10], F16, tag="p2t")
                nc.sync.dma_start(p1t[:], p1v)
                nc.sync.dma_start(p2t[:], p2v)

                abt = abp.tile([P, R, 20], F16, tag="ab")

                for src, vt, o in ((p1t, v1t, 0), (p2t, v2t, 10)):
                    for brow0, gs, ng in _batches_for(R):
                        K = gs * 10
                        tp4 = tpp.tile([128, 128 * GROUPS_PER_BATCH], F16,
                                       tag="tp")
                        mm4 = mmp.tile([P, 120 * GROUPS_PER_BATCH], F32,
                                       tag="mm")
                        for g in range(ng):
                            r0 = brow0 + g * gs
                            nc.tensor.transpose(
                                tp4[0:K, g * 128:(g + 1) * 128],
                                src[:, r0 * 10:(r0 + gs) * 10], idt[:])
                        pt4 = ptp.tile([120, 128 * GROUPS_PER_BATCH], F16,
                                       tag="pt")
                        nc.scalar.copy(pt4[0:K, 0:ng * 128],
                                       tp4[0:K, 0:ng * 128])
                        for g in range(ng):
                            nc.tensor.matmul(
                                mm4[:, g * 120:g * 120 + K],
                                pt4[0:K, g * 128:(g + 1) * 128],
                                vt[0:K, 0:K], start=True, stop=True)
                        # Ln fused into the PSUM->SBUF copy; out is the
                        # strided [row, col] view of abt
                        ln_in = mm4[:, 0:(ng - 1) * 120 + K].rearrange(
                            "p (g r c) -> p g r c", g=ng, c=10)
                        ln_out_v = abt[:, brow0:brow0 + ng * gs, o:o + 10]
                        ln_out = ln_out_v.rearrange(
                            "p (g r) c -> p g r c", g=ng)
                        nc.scalar.activation(
                            ln_out, ln_in,
                            mybir.ActivationFunctionType.Ln)

                # bin-major maxes: one instr per mirror-class; bin k's
                # region is st slots [10k, 10k+cnt)
                st = stp.tile([P, R, 190], F16, tag="st")
                for c in range(9):
                    cnt = c + 1
                    O = 10 * c
                    out = with_pair(st[:, :, O:O + cnt], 2, 180 - 20 * c)
                    in0 = with_pair(abt[:, :, 0:cnt], 2, 9 - c)
                    if c == 0:
                        in1 = with_pair(abt[:, :, 10:11], 2, 9)
                    else:
                        in1 = with_pair(abt[:, :, 10 + c:9:-1], 2, 9 - c)
                    nc.vector.tensor_tensor(out, in0, in1, AluOpType.max)
                # class 9 (bin 9, cnt 10)
                nc.vector.tensor_tensor(st[:, :, 90:100], abt[:, :, 0:10],
                                        abt[:, :, 19:9:-1], AluOpType.max)

                # in-place reversed-half folds (fp16) down to 2 slots/bin
                for c in range(2, 10):
                    cnt = c + 1 if c < 9 else 10
                    O = 10 * c
                    paired = c < 9
                    pstride = 180 - 20 * c
                    feng = nc.gpsimd if c in POOL_FOLD_CLASSES else nc.vector

                    def pv(sl):
                        ap = st[:, :, sl]
                        return with_pair(ap, 2, pstride) if paired else ap

                    n = cnt
                    while n > 2:
                        h = n // 2
                        in1 = with_pair(
                            st[:, :, O + n - 1:O + n - 1 - h:-1], 2,
                            pstride) if paired else \
                            st[:, :, O + n - 1:O + n - 1 - h:-1]
                        feng.tensor_tensor(pv(slice(O, O + h)),
                                           pv(slice(O, O + h)), in1,
                                           AluOpType.add)
                        n = h + (n & 1)
                # single batched final: lpt[k] = st[10k] + st[10k+1]
                # (cnt-1 bins k=0,18 pair with their pre-zeroed pad slot)
                lpt = lpp.tile([P, R, 19], F16, tag="lp")
                fin_eng = nc.gpsimd if FINAL_ON_POOL else nc.vector
                fin_eng.tensor_tensor(
                    lpt[:], st[:, :, 0:190:10], st[:, :, 1:190:10],
                    AluOpType.add)

                # w = exp(logP) on ACT, fp16
                wt = wp.tile([P, R, 19], F16, tag="w")
                nc.scalar.activation(
                    wt[:].rearrange("p r k -> p (r k)"),
                    lpt[:].rearrange("p r k -> p (r k)"),
                    mybir.ActivationFunctionType.Exp)
                # sum(w) via fp16 reversed-half folds into a scratch tile
                # (tensor_reduce has no DVE 2x mode; fold adds do)
                sct = sm.tile([P, R, 9], F16, tag="sc")
                sl1_eng = nc.gpsimd if SUM_L1_ON_POOL else nc.vector
                sl1_eng.tensor_tensor(sct[:], wt[:, :, 0:9],
                                      wt[:, :, 18:9:-1], AluOpType.add)
                n = 9
                while n > 2:
                    h = n // 2
                    nc.vector.tensor_tensor(
                        sct[:, :, 0:h], sct[:, :, 0:h],
                        sct[:, :, n - 1:n - 1 - h:-1], AluOpType.add)
                    n = h + (n & 1)
                # S = sct0 + sct1 + w9; denom = 19 + 1e-9 - S; r = 1/denom
                swt = sm.tile([P, R], F32, tag="S")
                nc.vector.scalar_tensor_tensor(
                    swt[:].unsqueeze(2), sct[:, :, 0:1], 0.0,
                    sct[:, :, 1:2], AluOpType.add, AluOpType.add)
                nc.vector.tensor_tensor(swt[:].unsqueeze(2),
                                        swt[:].unsqueeze(2),
                                        wt[:, :, 9:10], AluOpType.add)
                nc.vector.tensor_scalar(swt[:], swt[:], -1.0, 19.0 + 1e-9,
                                        AluOpType.mult, AluOpType.add)
                rt = sm.tile([P, R], F32, tag="r")
                nc.vector.reciprocal(rt[:], swt[:])
                # u = 1 - w in place: fp16 packed SBUF tensor_scalar gets
                # the DVE 4x perf mode, then y = u*r (fp32 out)
                nc.vector.tensor_scalar(
                    wt[:].rearrange("p r k -> p (r k)"),
                    wt[:].rearrange("p r k -> p (r k)"),
                    -1.0, 1.0, AluOpType.mult, AluOpType.add)
                yt = wp.tile([P, R, 19], F32, tag="y")
                r_b = rt[:].unsqueeze(2).broadcast_to((P, R, 19))
                y_eng = nc.gpsimd if Y_ON_POOL else nc.vector
                y_eng.tensor_tensor(yt[:], wt[:], r_b, AluOpType.mult)
                nc.sync.dma_start(yv, yt[:].rearrange("p r k -> p (r k)"))

    nc.insert_act_table_loads = lambda: None
    nc.finalize()
    return nc


def _host_consts(W1, W2):
    def mmn(W):
        W = W.astype(np.float32)
        lo = W.min(1, keepdims=True)
        hi = W.max(1, keepdims=True)
        return (W - lo) / (hi - lo + np.float32(1e-8))

    eye12 = np.eye(12, dtype=np.float32)
    v1b = np.kron(eye12, (np.float32(1.0) - mmn(W1))).astype(np.float16)
    v2b = np.kron(eye12, (np.float32(1.0) - mmn(W2))).astype(np.float16)
    ident = np.eye(128, dtype=np.float16)
    return v1b, v2b, ident


def kernel(p1, p2, W1, W2, mask=None, **_unused):
    from concourse.bass_utils import run_bass_kernel_spmd

    if 'nc' not in _CACHED:
        _CACHED['nc'] = _build_nc()
    nc = _CACHED['nc']

    v1b, v2b, ident = _host_consts(W1, W2)
    p1 = np.ascontiguousarray(p1, dtype=np.float16)
    p2 = np.ascontiguousarray(p2, dtype=np.float16)

    in_maps = []
    for c in range(N_CORES):
        sl = slice(c * RPC, (c + 1) * RPC)
        in_maps.append({
            "p1": p1[sl], "p2": p2[sl],
            "v1b": v1b, "v2b": v2b, "ident": ident,
        })
    res = run_bass_kernel_spmd(nc, in_maps, list(range(N_CORES)))
    out = np.concatenate([res.results[c]["y"] for c in range(N_CORES)], axis=0)
    return out.astype(np.float32)


if __name__ == "__main__":
    rng = np.random.default_rng(0)
    p1 = rng.random((B, 10), dtype=np.float32)
    p1 /= p1.sum(1, keepdims=True)
    p2 = rng.random((B, 10), dtype=np.float32)
    p2 /= p2.sum(1, keepdims=True)
    W1 = rng.random((10, 10), dtype=np.float32)
    W2 = rng.random((10, 10), dtype=np.float32)
    y = kernel(p1, p2, W1, W2)
    print("kernel ran, y shape", y.shape, "sum", float(y.sum()))
